# revision 32
# baseline (speedup 1.0000x reference)
"""Trainium2 Bass kernel for nn_MultiHeadAttention_22419729285517.

Reference computation (softmax-free multi-head attention):
    qkv = x @ w_qkv + b_qkv            # [B,N,3C] -> q,k,v  [B,H,N,D]
    attn = (q @ k^T) / sqrt(D)         # [B,H,N,N]  (NO softmax)
    out  = attn @ v                    # [B,H,N,D]
    out  = concat_heads(out) @ w_proj + b_proj

Because there is no softmax, attention is associative:
    (q @ k^T) @ v = q @ (k^T @ v)
so the N x N attention matrix never needs to exist.  Per head,
kv_h = k_h^T @ v_h is just [D,D] = [64,64].  Folding the output
projection in as well, the whole computation per batch b becomes

    out_b = q_b @ R_b + b_proj
    R_b[h*D+i, :] = sum_j kv_{b,h}[i,j] * w_proj[h*D+j, :]

Sharding (8 cores): sequence-parallel.  Core c owns rows
[s*1024,(s+1)*1024) of batch b, where b = c//4, s = c%4.  Each core:
  1. k,v = x_c @ w_kv                      (local rows, all heads)
  2. vk_h(partial) = v_h^T @ k_h           (= kv_h^T, partial over rows;
     heads processed two-at-a-time as 128x128 block matmuls)
  3. AllReduce vk over the 4 cores of the same batch
  4. q^T projection                        (overlaps the AllReduce)
  5. R rows = blockdiag(vk pair) @ w_proj row-pairs
  6. outT = R^T-as-lhsT @ q^T   -> [768, 1024] f32 (transposed; host
     transposes back — keeps every matmul at the max 512 moving dim)
The 1/sqrt(D) = 0.125 scale is folded into w_q on the host (exact in
bf16: power of two).  b_proj is added on the host (free, general).
All matmuls run in bf16 with fp32 PSUM accumulation (fp32 matmul is 2x
slower on PE); host pre-casts inputs to bf16.
"""

import numpy as np
import ml_dtypes

import concourse.bass as bass
import concourse.mybir as mybir
from concourse import bacc, tile
from concourse import bass_utils

BF16 = mybir.dt.bfloat16
F32 = mybir.dt.float32

B, N, C = 2, 4096, 768
H, D = 12, 64
NCORES = 8
ROWS = (B * N) // NCORES  # 1024 rows per core
KT = C // 128  # 6 contraction tiles of 128
MT = ROWS // 128  # 8 row tiles per core
NP_ = H // 2  # 6 head pairs
NB = ml_dtypes.bfloat16


def _emit_body(nc, tc, pools, tensors, rep, use_collective=True):
    """One full computation pass. rep: unique suffix for tile names."""
    wpool, apool, psum, psum_vk, opool, dram = pools
    x_in, xT, wk, wv, wq, wproj, out = tensors
    replica_groups = [[0, 1, 2, 3], [4, 5, 6, 7]]

    # ---- load inputs to SBUF (x first: the Gram phase needs it) ----
    x_sb, xT_sb, wk_sb, wv_sb, wq_sb, wproj_sb = [], [], [], [], [], []
    for m in range(MT):
        xm = apool.tile([128, C], BF16, name=f"x_m{m}_{rep}", tag=f"x_m{m}", bufs=2)
        if m == 0:
            # split so the first G matmul's operands arrive sooner
            nc.sync.dma_start(xm[:, :512], x_in[0:128, 0:512])
            nc.sync.dma_start(xm[:, 512:], x_in[0:128, 512:C])
        else:
            nc.sync.dma_start(xm[:], x_in[m * 128 : (m + 1) * 128, :])
        x_sb.append(xm)
    for kt in range(KT):
        wk_t = wpool.tile([128, C], BF16, name=f"wk_t{kt}_{rep}", tag=f"wk_t{kt}")
        nc.sync.dma_start(wk_t[:], wk[kt * 128 : (kt + 1) * 128, :])
        wk_sb.append(wk_t)
        x_t = apool.tile(
            [128, ROWS], BF16, name=f"x_t{kt}_{rep}", tag=f"x_t{kt}", bufs=2
        )
        nc.sync.dma_start(x_t[:], xT[kt * 128 : (kt + 1) * 128, :])
        xT_sb.append(x_t)
    for kt in range(KT):
        wv_t = wpool.tile([128, C], BF16, name=f"wv_t{kt}_{rep}", tag=f"wv_t{kt}")
        nc.sync.dma_start(wv_t[:], wv[kt * 128 : (kt + 1) * 128, :])
        wv_sb.append(wv_t)
        wq_t = wpool.tile([128, C], BF16, name=f"wq_t{kt}_{rep}", tag=f"wq_t{kt}")
        nc.sync.dma_start(wq_t[:], wq[kt * 128 : (kt + 1) * 128, :])
        wq_sb.append(wq_t)
    for p in range(NP_):
        wp_t = wpool.tile([128, C], BF16, name=f"wp_t{p}_{rep}", tag=f"wp_t{p}")
        nc.sync.dma_start(wp_t[:], wproj[p * 128 : (p + 1) * 128, :])
        wproj_sb.append(wp_t)

    # ---- phase 1: local Gram matrix G = x_c^T x_c  [768, 768] bf16 ----
    # k,v are only ever used through vk_h = v_h^T k_h = Wv_h^T G Wk_h, so
    # k,v themselves are never materialized.  G is symmetric (and exactly
    # so after rounding: G[a,b] and G[b,a] share the same f32 sum order),
    # which lets G tiles serve directly as their own transposed lhsT.
    G_sb = [
        apool.tile([128, C], BF16, name=f"g_t{it}_{rep}", tag=f"g_t{it}")
        for it in range(KT)
    ]
    for it in range(KT):
        ps = psum.tile([128, C], F32, name="ps_g", tag="mm")
        for m in range(MT):  # stationary x[m][:,it] reused across j chunks
            for j0, jn in ((0, 512), (512, 256)):
                nc.tensor.matmul(
                    ps[:, j0 : j0 + jn],
                    x_sb[m][:, it * 128 : (it + 1) * 128],
                    x_sb[m][:, j0 : j0 + jn],
                    start=(m == 0),
                    stop=(m == MT - 1),
                )
        if it % 2 == 0:
            nc.vector.tensor_copy(G_sb[it][:], ps[:])
        else:
            nc.scalar.copy(G_sb[it][:], ps[:])

    # ---- phase 1b: GWk = G @ w_k  [768, 768] bf16 ----
    GWk_sb = [
        apool.tile([128, C], BF16, name=f"gwk_t{at}_{rep}", tag=f"gwk_t{at}")
        for at in range(KT)
    ]
    for at in range(KT):
        ps = psum.tile([128, C], F32, name="ps_gwk", tag="mm")
        for bt in range(KT):  # lhsT = G[bt][:, at] == G^T block by symmetry
            for i0, inn in ((0, 512), (512, 256)):
                nc.tensor.matmul(
                    ps[:, i0 : i0 + inn],
                    G_sb[bt][:, at * 128 : (at + 1) * 128],
                    wk_sb[bt][:, i0 : i0 + inn],
                    start=(bt == 0),
                    stop=(bt == KT - 1),
                )
        if at % 2 == 0:
            nc.vector.tensor_copy(GWk_sb[at][:], ps[:])
        else:
            nc.scalar.copy(GWk_sb[at][:], ps[:])

    # ---- phase 2: vk pair-blocks = Wv-pair^T @ GWk-pair-cols ----
    # pair p = heads (2p, 2p+1): psum block [128, 128] whose diag 64x64
    # sub-blocks are vk_{2p} and vk_{2p+1}; off-diag cross-head garbage
    # is never copied out (strided diag extraction below)
    ps_vk = [
        psum_vk.tile([128, 384], F32, name=f"ps_vk{g}", tag=f"vk{g}")
        for g in range(2)
    ]
    for p in range(NP_):
        ps = ps_vk[p // 3]
        col = (p % 3) * 128
        for at in range(KT):
            nc.tensor.matmul(
                ps[:, col : col + 128],
                wv_sb[at][:, p * 128 : (p + 1) * 128],  # Wv pair cols
                GWk_sb[at][:, p * 128 : (p + 1) * 128],  # GWk pair cols
                start=(at == 0),
                stop=(at == KT - 1),
            )
    # vk_sb [128, 384] bf16: col block p holds the pair's diag 64x64
    # blocks only (partitions 0:64 = vk_{2p}, 64:128 = vk_{2p+1}),
    # extracted from the psum pair-blocks with strided casting copies —
    # the off-diag cross-head products are never copied out
    vk_sb = apool.tile([128, 384], BF16, name=f"vk_sb_{rep}", tag="vk_sb")
    for g in range(2):
        ps3 = ps_vk[g].rearrange("p (pr s) -> p pr s", s=128)
        dst = vk_sb[:, g * 192 : (g + 1) * 192].rearrange(
            "p (pr d) -> p pr d", d=64
        )
        nc.vector.tensor_copy(dst[0:64], ps3[0:64, :, 0:64])
        nc.vector.tensor_copy(dst[64:128], ps3[64:128, :, 64:128])

    # ---- phase 3: AllReduce vk (bf16, 96 KB) over the 4-core group ----
    vkr = apool.tile([128, 384], BF16, name=f"vkr_{rep}", tag="vkr")
    if use_collective:
        cc_in = dram.tile([128, 384], BF16, name=f"cc_in_{rep}", tag="cc_in")
        cc_out = dram.tile([128, 384], BF16, name=f"cc_out_{rep}", tag="cc_out")
        nc.sync.dma_start(cc_in[:], vk_sb[:])
        nc.gpsimd.collective_compute(
            "AllReduce",
            mybir.AluOpType.add,
            replica_groups=replica_groups,
            ins=[cc_in.opt()],
            outs=[cc_out.opt()],
        )
        nc.sync.dma_start(vkr[:], cc_out[:])
    else:
        nc.vector.tensor_copy(vkr[:], vk_sb[:])

    # ---- phase 4: q^T -> qT_sb[t] [128,1024] (overlaps the AllReduce) ----
    qT_sb = [
        apool.tile([128, ROWS], BF16, name=f"q_t{t}_{rep}", tag=f"q_t{t}")
        for t in range(KT)
    ]
    for t in range(KT):
        ps = psum.tile([128, ROWS], F32, name="ps_q", tag="mm")
        for kt in range(KT):  # stationary wq[kt][:,t] reused across mc
            for mc in range(ROWS // 512):
                nc.tensor.matmul(
                    ps[:, mc * 512 : (mc + 1) * 512],
                    wq_sb[kt][:, t * 128 : (t + 1) * 128],
                    xT_sb[kt][:, mc * 512 : (mc + 1) * 512],
                    start=(kt == 0),
                    stop=(kt == KT - 1),
                )
        if t % 2 == 0:
            nc.vector.tensor_copy(qT_sb[t][:], ps[:])
        else:
            nc.scalar.copy(qT_sb[t][:], ps[:])

    # ---- phase 5: R row-pairs = blockdiag(vk pair) @ w_proj row-pair ----
    R_sb = [
        apool.tile([128, C], BF16, name=f"r_t{p}_{rep}", tag=f"r_t{p}")
        for p in range(NP_)
    ]
    for p in range(NP_):
        ps = psum.tile([128, C], F32, name="ps_r", tag="mm")
        for n0, nn in ((0, 512), (512, 256)):
            # even head of the pair: partitions 0:64 of psum
            nc.tensor.matmul(
                ps[0:64, n0 : n0 + nn],
                vkr[0:64, p * 64 : (p + 1) * 64],
                wproj_sb[p][0:64, n0 : n0 + nn],
                start=True,
                stop=True,
            )
            # odd head: partitions 64:128 (lhsT/rhs/out all base 64)
            nc.tensor.matmul(
                ps[64:128, n0 : n0 + nn],
                vkr[64:128, p * 64 : (p + 1) * 64],
                wproj_sb[p][64:128, n0 : n0 + nn],
                start=True,
                stop=True,
            )
        if p % 2 == 0:
            nc.vector.tensor_copy(R_sb[p][:], ps[:])
        else:
            nc.scalar.copy(R_sb[p][:], ps[:])

    # ---- phase 6: outT = R-as-lhsT @ qT  -> [768, 1024] (transposed) ----
    for nt in range(KT):  # 6 output col tiles of 128 (C dim)
        o_t = opool.tile([128, ROWS], F32, name="o_t", tag="o_t")
        ps = psum.tile([128, ROWS], F32, name="ps_o", tag="mm")
        for dt in range(KT):  # stationary R[dt][:,nt] reused across mc
            for mc in range(ROWS // 512):
                nc.tensor.matmul(
                    ps[:, mc * 512 : (mc + 1) * 512],
                    R_sb[dt][:, nt * 128 : (nt + 1) * 128],
                    qT_sb[dt][:, mc * 512 : (mc + 1) * 512],
                    start=(dt == 0),
                    stop=(dt == KT - 1),
                )
        if nt % 2 == 0:
            nc.vector.tensor_copy(o_t[:], ps[:])
        else:
            nc.scalar.copy(o_t[:], ps[:])
        nc.gpsimd.dma_start(out[nt * 128 : (nt + 1) * 128, :], o_t[:])


def _build_kernel(repeat=1, use_collective=True, num_devices=NCORES):
    nc = bacc.Bacc(
        "TRN2", target_bir_lowering=False, debug=False, num_devices=num_devices
    )

    x_in = nc.dram_tensor("x", [ROWS, C], BF16, kind="ExternalInput")
    xT = nc.dram_tensor("xT", [C, ROWS], BF16, kind="ExternalInput")
    wk = nc.dram_tensor("wk", [C, C], BF16, kind="ExternalInput")
    wv = nc.dram_tensor("wv", [C, C], BF16, kind="ExternalInput")
    wq = nc.dram_tensor("wq", [C, C], BF16, kind="ExternalInput")
    wproj = nc.dram_tensor("wproj", [C, C], BF16, kind="ExternalInput")
    # transposed output [C, ROWS]; host transposes back
    out = nc.dram_tensor("out", [C, ROWS], F32, kind="ExternalOutput")

    with tile.TileContext(nc) as tc:
        with (
            tc.tile_pool(name="weights", bufs=2) as wpool,
            tc.tile_pool(name="acts", bufs=1) as apool,
            tc.tile_pool(name="psum", bufs=2, space="PSUM") as psum,
            tc.tile_pool(name="psum_vk", bufs=1, space="PSUM") as psum_vk,
            tc.tile_pool(name="outp", bufs=3) as opool,
            tc.tile_pool(name="dram", bufs=2, space="DRAM") as dram,
        ):
            pools = (wpool, apool, psum, psum_vk, opool, dram)
            tensors = (x_in, xT, wk, wv, wq, wproj, out)
            for rep in range(repeat):
                _emit_body(nc, tc, pools, tensors, rep, use_collective)

    nc.compile()
    return nc


_NC_CACHE = None


def _get_nc():
    global _NC_CACHE
    if _NC_CACHE is None:
        _NC_CACHE = _build_kernel()
    return _NC_CACHE


def _numpy_fallback(x, w_qkv, b_qkv, w_proj, b_proj):
    qkv = (x @ w_qkv + b_qkv).reshape(B, N, 3, H, D).transpose(2, 0, 3, 1, 4)
    q, k, v = qkv[0], qkv[1], qkv[2]
    out = np.zeros((B, N, C), np.float32)
    for b in range(B):
        for h in range(H):
            kv = k[b, h].T @ v[b, h]
            out[b, :, h * D : (h + 1) * D] = (q[b, h] / np.sqrt(D)) @ kv
    return out @ w_proj + b_proj


def _make_in_maps(x, w_qkv, w_proj):
    wq_np = np.ascontiguousarray((w_qkv[:, :C] * 0.125)).astype(NB)
    wk_np = np.ascontiguousarray(w_qkv[:, C : 2 * C]).astype(NB)
    wv_np = np.ascontiguousarray(w_qkv[:, 2 * C :]).astype(NB)
    wproj_np = np.ascontiguousarray(w_proj).astype(NB)
    x2 = np.asarray(x, np.float32).reshape(B * N, C)
    in_maps = []
    for c in range(NCORES):
        xc = x2[c * ROWS : (c + 1) * ROWS, :]
        x_np = np.ascontiguousarray(xc).astype(NB)
        xT_np = np.ascontiguousarray(xc.T).astype(NB)
        in_maps.append(
            {
                "x": x_np,
                "xT": xT_np,
                "wk": wk_np,
                "wv": wv_np,
                "wq": wq_np,
                "wproj": wproj_np,
            }
        )
    return in_maps


def kernel(x, w_qkv, b_qkv, w_proj, b_proj, **_kwargs):
    x = np.ascontiguousarray(x, dtype=np.float32)
    w_qkv = np.asarray(w_qkv, dtype=np.float32)
    b_qkv = np.asarray(b_qkv, dtype=np.float32)
    w_proj = np.asarray(w_proj, dtype=np.float32)
    b_proj = np.asarray(b_proj, dtype=np.float32)

    if np.abs(b_qkv).max() != 0:
        # problem spec fills b_qkv with zeros; keep a general fallback
        return _numpy_fallback(x, w_qkv, b_qkv, w_proj, b_proj).astype(np.float32)

    in_maps = _make_in_maps(x, w_qkv, w_proj)
    nc = _get_nc()
    res = bass_utils.run_bass_kernel_spmd(
        nc, in_maps, core_ids=list(range(NCORES))
    )
    out = np.empty((B * N, C), np.float32)
    for c in range(NCORES):
        out[c * ROWS : (c + 1) * ROWS, :] = res.results[c]["out"].T
    out = out.reshape(B, N, C)
    if np.abs(b_proj).max() != 0:
        out = out + b_proj
    return out.astype(np.float32)


if __name__ == "__main__":
    rng = np.random.default_rng(0)
    inputs = {
        "x": rng.standard_normal((B, N, C), dtype=np.float32),
        "w_qkv": (rng.standard_normal((C, 3 * C)) * 0.02).astype(np.float32),
        "b_qkv": np.zeros((3 * C,), np.float32),
        "w_proj": (rng.standard_normal((C, C)) * 0.02).astype(np.float32),
        "b_proj": np.zeros((C,), np.float32),
    }
    got = kernel(**inputs)
    want = _numpy_fallback(**inputs)
    err = np.linalg.norm(got - want) / np.linalg.norm(want)
    print("rel l2 err vs numpy:", err)


# revision 35
# speedup vs baseline: 1.0180x; 1.0180x over previous
"""Trainium2 Bass kernel for nn_MultiHeadAttention_22419729285517.

Reference computation (softmax-free multi-head attention):
    qkv = x @ w_qkv + b_qkv            # [B,N,3C] -> q,k,v  [B,H,N,D]
    attn = (q @ k^T) / sqrt(D)         # [B,H,N,N]  (NO softmax)
    out  = attn @ v                    # [B,H,N,D]
    out  = concat_heads(out) @ w_proj + b_proj

Because there is no softmax, attention is associative:
    (q @ k^T) @ v = q @ (k^T @ v)
so the N x N attention matrix never needs to exist.  Per head,
kv_h = k_h^T @ v_h is just [D,D] = [64,64].  Folding the output
projection in as well, the whole computation per batch b becomes

    out_b = q_b @ R_b + b_proj
    R_b[h*D+i, :] = sum_j kv_{b,h}[i,j] * w_proj[h*D+j, :]

Sharding (8 cores): sequence-parallel.  Core c owns rows
[s*1024,(s+1)*1024) of batch b, where b = c//4, s = c%4.  Each core:
  1. k,v = x_c @ w_kv                      (local rows, all heads)
  2. vk_h(partial) = v_h^T @ k_h           (= kv_h^T, partial over rows;
     heads processed two-at-a-time as 128x128 block matmuls)
  3. AllReduce vk over the 4 cores of the same batch
  4. q^T projection                        (overlaps the AllReduce)
  5. R rows = blockdiag(vk pair) @ w_proj row-pairs
  6. outT = R^T-as-lhsT @ q^T   -> [768, 1024] f32 (transposed; host
     transposes back — keeps every matmul at the max 512 moving dim)
The 1/sqrt(D) = 0.125 scale is folded into w_q on the host (exact in
bf16: power of two).  b_proj is added on the host (free, general).
All matmuls run in bf16 with fp32 PSUM accumulation (fp32 matmul is 2x
slower on PE); host pre-casts inputs to bf16.
"""

import numpy as np
import ml_dtypes

import concourse.bass as bass
import concourse.mybir as mybir
from concourse import bacc, tile
from concourse import bass_utils

BF16 = mybir.dt.bfloat16
F32 = mybir.dt.float32

B, N, C = 2, 4096, 768
H, D = 12, 64
NCORES = 8
ROWS = (B * N) // NCORES  # 1024 rows per core
KT = C // 128  # 6 contraction tiles of 128
MT = ROWS // 128  # 8 row tiles per core
NP_ = H // 2  # 6 head pairs
NB = ml_dtypes.bfloat16


def _emit_body(nc, tc, pools, tensors, rep, use_collective=True):
    """One full computation pass. rep: unique suffix for tile names."""
    wpool, apool, psum, psum_vk, opool, dram = pools
    x_in, xT, wk, wv, wq, wproj, out = tensors
    replica_groups = [[0, 1, 2, 3], [4, 5, 6, 7]]

    # ---- load inputs to SBUF (x first: the Gram phase needs it) ----
    x_sb, xT_sb, wk_sb, wv_sb, wq_sb, wproj_sb = [], [], [], [], [], []
    for m in range(MT):
        xm = apool.tile([128, C], BF16, name=f"x_m{m}_{rep}", tag=f"x_m{m}", bufs=2)
        if m == 0:
            # split so the first G matmul's operands arrive sooner
            nc.sync.dma_start(xm[:, :512], x_in[0:128, 0:512])
            nc.sync.dma_start(xm[:, 512:], x_in[0:128, 512:C])
        else:
            nc.sync.dma_start(xm[:], x_in[m * 128 : (m + 1) * 128, :])
        x_sb.append(xm)
    for kt in range(KT):
        wk_t = wpool.tile([128, C], BF16, name=f"wk_t{kt}_{rep}", tag=f"wk_t{kt}")
        nc.sync.dma_start(wk_t[:], wk[kt * 128 : (kt + 1) * 128, :])
        wk_sb.append(wk_t)
        x_t = apool.tile(
            [128, ROWS], BF16, name=f"x_t{kt}_{rep}", tag=f"x_t{kt}", bufs=2
        )
        nc.sync.dma_start(x_t[:], xT[kt * 128 : (kt + 1) * 128, :])
        xT_sb.append(x_t)
    for kt in range(KT):
        wv_t = wpool.tile([128, C], BF16, name=f"wv_t{kt}_{rep}", tag=f"wv_t{kt}")
        nc.sync.dma_start(wv_t[:], wv[kt * 128 : (kt + 1) * 128, :])
        wv_sb.append(wv_t)
        wq_t = wpool.tile([128, C], BF16, name=f"wq_t{kt}_{rep}", tag=f"wq_t{kt}")
        nc.sync.dma_start(wq_t[:], wq[kt * 128 : (kt + 1) * 128, :])
        wq_sb.append(wq_t)
    for p in range(NP_):
        wp_t = wpool.tile([128, C], BF16, name=f"wp_t{p}_{rep}", tag=f"wp_t{p}")
        nc.sync.dma_start(wp_t[:], wproj[p * 128 : (p + 1) * 128, :])
        wproj_sb.append(wp_t)

    # ---- phase 1: local Gram matrix G = x_c^T x_c  [768, 768] bf16 ----
    # k,v are only ever used through vk_h = v_h^T k_h = Wv_h^T G Wk_h, so
    # k,v themselves are never materialized.  G is symmetric (and exactly
    # so after rounding: G[a,b] and G[b,a] share the same f32 sum order),
    # which lets G tiles serve directly as their own transposed lhsT.
    G_sb = [
        apool.tile([128, C], BF16, name=f"g_t{it}_{rep}", tag=f"g_t{it}")
        for it in range(KT)
    ]
    for it in range(KT):
        ps = psum.tile([128, C], F32, name="ps_g", tag="mm")
        for m in range(MT):  # stationary x[m][:,it] reused across j chunks
            for j0, jn in ((0, 512), (512, 256)):
                nc.tensor.matmul(
                    ps[:, j0 : j0 + jn],
                    x_sb[m][:, it * 128 : (it + 1) * 128],
                    x_sb[m][:, j0 : j0 + jn],
                    start=(m == 0),
                    stop=(m == MT - 1),
                )
        if it % 2 == 0:
            nc.vector.tensor_copy(G_sb[it][:], ps[:])
        else:
            nc.scalar.copy(G_sb[it][:], ps[:])

    # ---- phase 1b: GWk = G @ w_k  [768, 768] bf16 ----
    GWk_sb = [
        apool.tile([128, C], BF16, name=f"gwk_t{at}_{rep}", tag=f"gwk_t{at}")
        for at in range(KT)
    ]
    for at in range(KT):
        ps = psum.tile([128, C], F32, name="ps_gwk", tag="mm")
        for bt in range(KT):  # lhsT = G[bt][:, at] == G^T block by symmetry
            for i0, inn in ((0, 512), (512, 256)):
                nc.tensor.matmul(
                    ps[:, i0 : i0 + inn],
                    G_sb[bt][:, at * 128 : (at + 1) * 128],
                    wk_sb[bt][:, i0 : i0 + inn],
                    start=(bt == 0),
                    stop=(bt == KT - 1),
                )
        if at % 2 == 0:
            nc.vector.tensor_copy(GWk_sb[at][:], ps[:])
        else:
            nc.scalar.copy(GWk_sb[at][:], ps[:])

    # ---- phase 2: vk pair-blocks = Wv-pair^T @ GWk-pair-cols ----
    # pair p = heads (2p, 2p+1): psum block [128, 128] whose diag 64x64
    # sub-blocks are vk_{2p} and vk_{2p+1}; off-diag cross-head garbage
    # is never copied out (strided diag extraction below)
    ps_vk = [
        psum_vk.tile([128, 384], F32, name=f"ps_vk{g}", tag=f"vk{g}")
        for g in range(2)
    ]
    for p in range(NP_):
        ps = ps_vk[p // 3]
        col = (p % 3) * 128
        for at in range(KT):
            nc.tensor.matmul(
                ps[:, col : col + 128],
                wv_sb[at][:, p * 128 : (p + 1) * 128],  # Wv pair cols
                GWk_sb[at][:, p * 128 : (p + 1) * 128],  # GWk pair cols
                start=(at == 0),
                stop=(at == KT - 1),
            )
    # vk_sb [128, 384] bf16: col block p holds the pair's diag 64x64
    # blocks only (partitions 0:64 = vk_{2p}, 64:128 = vk_{2p+1}),
    # extracted from the psum pair-blocks with strided casting copies —
    # the off-diag cross-head products are never copied out
    vk_sb = apool.tile([128, 384], BF16, name=f"vk_sb_{rep}", tag="vk_sb")
    for g in range(2):
        ps3 = ps_vk[g].rearrange("p (pr s) -> p pr s", s=128)
        dst = vk_sb[:, g * 192 : (g + 1) * 192].rearrange(
            "p (pr d) -> p pr d", d=64
        )
        nc.vector.tensor_copy(dst[0:64], ps3[0:64, :, 0:64])
        nc.vector.tensor_copy(dst[64:128], ps3[64:128, :, 64:128])

    # ---- phase 3: AllReduce vk (bf16, 96 KB) over the 4-core group ----
    vkr = apool.tile([128, 384], BF16, name=f"vkr_{rep}", tag="vkr")
    if use_collective:
        cc_in = dram.tile([128, 384], BF16, name=f"cc_in_{rep}", tag="cc_in")
        cc_out = dram.tile([128, 384], BF16, name=f"cc_out_{rep}", tag="cc_out")
        # scalar-engine DMA queue: keeps the collective's bounce hops off
        # the sync queue, which is busy draining the big input loads
        nc.scalar.dma_start(cc_in[:], vk_sb[:])
        nc.gpsimd.collective_compute(
            "AllReduce",
            mybir.AluOpType.add,
            replica_groups=replica_groups,
            ins=[cc_in.opt()],
            outs=[cc_out.opt()],
        )
        nc.scalar.dma_start(vkr[:], cc_out[:])
    else:
        nc.vector.tensor_copy(vkr[:], vk_sb[:])

    # ---- phase 4: q^T -> qT_sb[t] [128,1024] (overlaps the AllReduce) ----
    qT_sb = [
        apool.tile([128, ROWS], BF16, name=f"q_t{t}_{rep}", tag=f"q_t{t}")
        for t in range(KT)
    ]
    for t in range(KT):
        ps = psum.tile([128, ROWS], F32, name="ps_q", tag="mm")
        for kt in range(KT):  # stationary wq[kt][:,t] reused across mc
            for mc in range(ROWS // 512):
                nc.tensor.matmul(
                    ps[:, mc * 512 : (mc + 1) * 512],
                    wq_sb[kt][:, t * 128 : (t + 1) * 128],
                    xT_sb[kt][:, mc * 512 : (mc + 1) * 512],
                    start=(kt == 0),
                    stop=(kt == KT - 1),
                )
        if t % 2 == 0:
            nc.vector.tensor_copy(qT_sb[t][:], ps[:])
        else:
            nc.scalar.copy(qT_sb[t][:], ps[:])

    # ---- phase 5: R row-pairs = blockdiag(vk pair) @ w_proj row-pair ----
    R_sb = [
        apool.tile([128, C], BF16, name=f"r_t{p}_{rep}", tag=f"r_t{p}")
        for p in range(NP_)
    ]
    for p in range(NP_):
        ps = psum.tile([128, C], F32, name="ps_r", tag="mm")
        for n0, nn in ((0, 512), (512, 256)):
            # even head of the pair: partitions 0:64 of psum
            nc.tensor.matmul(
                ps[0:64, n0 : n0 + nn],
                vkr[0:64, p * 64 : (p + 1) * 64],
                wproj_sb[p][0:64, n0 : n0 + nn],
                start=True,
                stop=True,
            )
            # odd head: partitions 64:128 (lhsT/rhs/out all base 64)
            nc.tensor.matmul(
                ps[64:128, n0 : n0 + nn],
                vkr[64:128, p * 64 : (p + 1) * 64],
                wproj_sb[p][64:128, n0 : n0 + nn],
                start=True,
                stop=True,
            )
        if p % 2 == 0:
            nc.vector.tensor_copy(R_sb[p][:], ps[:])
        else:
            nc.scalar.copy(R_sb[p][:], ps[:])

    # ---- phase 6: outT = R-as-lhsT @ qT  -> [768, 1024] (transposed) ----
    for nt in range(KT):  # 6 output col tiles of 128 (C dim)
        o_t = opool.tile([128, ROWS], F32, name="o_t", tag="o_t")
        ps = psum.tile([128, ROWS], F32, name="ps_o", tag="mm")
        for dt in range(KT):  # stationary R[dt][:,nt] reused across mc
            for mc in range(ROWS // 512):
                nc.tensor.matmul(
                    ps[:, mc * 512 : (mc + 1) * 512],
                    R_sb[dt][:, nt * 128 : (nt + 1) * 128],
                    qT_sb[dt][:, mc * 512 : (mc + 1) * 512],
                    start=(dt == 0),
                    stop=(dt == KT - 1),
                )
        if nt % 2 == 0:
            nc.vector.tensor_copy(o_t[:], ps[:])
        else:
            nc.scalar.copy(o_t[:], ps[:])
        nc.gpsimd.dma_start(out[nt * 128 : (nt + 1) * 128, :], o_t[:])


def _build_kernel(repeat=1, use_collective=True, num_devices=NCORES):
    nc = bacc.Bacc(
        "TRN2", target_bir_lowering=False, debug=False, num_devices=num_devices
    )

    x_in = nc.dram_tensor("x", [ROWS, C], BF16, kind="ExternalInput")
    xT = nc.dram_tensor("xT", [C, ROWS], BF16, kind="ExternalInput")
    wk = nc.dram_tensor("wk", [C, C], BF16, kind="ExternalInput")
    wv = nc.dram_tensor("wv", [C, C], BF16, kind="ExternalInput")
    wq = nc.dram_tensor("wq", [C, C], BF16, kind="ExternalInput")
    wproj = nc.dram_tensor("wproj", [C, C], BF16, kind="ExternalInput")
    # transposed output [C, ROWS]; host transposes back
    out = nc.dram_tensor("out", [C, ROWS], F32, kind="ExternalOutput")

    with tile.TileContext(nc) as tc:
        with (
            tc.tile_pool(name="weights", bufs=2) as wpool,
            tc.tile_pool(name="acts", bufs=1) as apool,
            tc.tile_pool(name="psum", bufs=2, space="PSUM") as psum,
            tc.tile_pool(name="psum_vk", bufs=1, space="PSUM") as psum_vk,
            tc.tile_pool(name="outp", bufs=3) as opool,
            tc.tile_pool(name="dram", bufs=2, space="DRAM") as dram,
        ):
            pools = (wpool, apool, psum, psum_vk, opool, dram)
            tensors = (x_in, xT, wk, wv, wq, wproj, out)
            for rep in range(repeat):
                _emit_body(nc, tc, pools, tensors, rep, use_collective)

    nc.compile()
    return nc


_NC_CACHE = None


def _get_nc():
    global _NC_CACHE
    if _NC_CACHE is None:
        _NC_CACHE = _build_kernel()
    return _NC_CACHE


def _numpy_fallback(x, w_qkv, b_qkv, w_proj, b_proj):
    qkv = (x @ w_qkv + b_qkv).reshape(B, N, 3, H, D).transpose(2, 0, 3, 1, 4)
    q, k, v = qkv[0], qkv[1], qkv[2]
    out = np.zeros((B, N, C), np.float32)
    for b in range(B):
        for h in range(H):
            kv = k[b, h].T @ v[b, h]
            out[b, :, h * D : (h + 1) * D] = (q[b, h] / np.sqrt(D)) @ kv
    return out @ w_proj + b_proj


def _make_in_maps(x, w_qkv, w_proj):
    wq_np = np.ascontiguousarray((w_qkv[:, :C] * 0.125)).astype(NB)
    wk_np = np.ascontiguousarray(w_qkv[:, C : 2 * C]).astype(NB)
    wv_np = np.ascontiguousarray(w_qkv[:, 2 * C :]).astype(NB)
    wproj_np = np.ascontiguousarray(w_proj).astype(NB)
    x2 = np.asarray(x, np.float32).reshape(B * N, C)
    in_maps = []
    for c in range(NCORES):
        xc = x2[c * ROWS : (c + 1) * ROWS, :]
        x_np = np.ascontiguousarray(xc).astype(NB)
        xT_np = np.ascontiguousarray(xc.T).astype(NB)
        in_maps.append(
            {
                "x": x_np,
                "xT": xT_np,
                "wk": wk_np,
                "wv": wv_np,
                "wq": wq_np,
                "wproj": wproj_np,
            }
        )
    return in_maps


def kernel(x, w_qkv, b_qkv, w_proj, b_proj, **_kwargs):
    x = np.ascontiguousarray(x, dtype=np.float32)
    w_qkv = np.asarray(w_qkv, dtype=np.float32)
    b_qkv = np.asarray(b_qkv, dtype=np.float32)
    w_proj = np.asarray(w_proj, dtype=np.float32)
    b_proj = np.asarray(b_proj, dtype=np.float32)

    if np.abs(b_qkv).max() != 0:
        # problem spec fills b_qkv with zeros; keep a general fallback
        return _numpy_fallback(x, w_qkv, b_qkv, w_proj, b_proj).astype(np.float32)

    in_maps = _make_in_maps(x, w_qkv, w_proj)
    nc = _get_nc()
    res = bass_utils.run_bass_kernel_spmd(
        nc, in_maps, core_ids=list(range(NCORES))
    )
    out = np.empty((B * N, C), np.float32)
    for c in range(NCORES):
        out[c * ROWS : (c + 1) * ROWS, :] = res.results[c]["out"].T
    out = out.reshape(B, N, C)
    if np.abs(b_proj).max() != 0:
        out = out + b_proj
    return out.astype(np.float32)


if __name__ == "__main__":
    rng = np.random.default_rng(0)
    inputs = {
        "x": rng.standard_normal((B, N, C), dtype=np.float32),
        "w_qkv": (rng.standard_normal((C, 3 * C)) * 0.02).astype(np.float32),
        "b_qkv": np.zeros((3 * C,), np.float32),
        "w_proj": (rng.standard_normal((C, C)) * 0.02).astype(np.float32),
        "b_proj": np.zeros((C,), np.float32),
    }
    got = kernel(**inputs)
    want = _numpy_fallback(**inputs)
    err = np.linalg.norm(got - want) / np.linalg.norm(want)
    print("rel l2 err vs numpy:", err)
